# revision 3
# baseline (speedup 1.0000x reference)
"""Walsh-Hadamard transform (1024-pt) * scale + shift on 8 Trainium2 cores.

Full inputs in, full output out. Data-parallel: 65536 rows of 1024 are split
8192/core. Per tile of 256 rows the on-core pipeline is 6 TensorE passes:
  A: transpose 128x128 chunks            -> elements(low7) on partitions
  B: matmul kron(H8,I16) (elem bits 6-4) -> layout preserved
  C: transpose back                      -> row-major intermediate
  D: strided-gather transpose            -> (bits 9-7, 3-0) on partitions
  E: matmul H128 (elem bits 9-7 + 3-0)
  F: transpose back + scatter copy       -> row-major output
H1024 = H8(b9-7) x H8(b6-4) x H16(b3-0); scale/32 folds into pass-B weights
when scale is uniform (general scale/shift applied on host, correctness path).
"""
import sys
import numpy as np

sys.path.insert(0, "/opt/trn_rl_repo")

ROWS_PER_CORE = 8192
SIZE = 1024
N_CORES = 8
TILE_ROWS = 256  # 2 row-slots x 128 partitions
N_TILES = ROWS_PER_CORE // TILE_ROWS

_CACHE = {}


def _hadamard(n):
    h = np.array([[1.0]])
    while h.shape[0] < n:
        h = np.block([[h, h], [h, -h]])
    return h


def _build():
    import concourse.bacc as bacc
    import concourse.mybir as mybir
    from concourse import tile

    F32 = mybir.dt.float32
    nc = bacc.Bacc("TRN2", target_bir_lowering=False, debug=False)
    x = nc.dram_tensor("x", [ROWS_PER_CORE, SIZE], F32, kind="ExternalInput").ap()
    w1 = nc.dram_tensor("w1", [128, 128], F32, kind="ExternalInput").ap()
    w2 = nc.dram_tensor("w2", [128, 128], F32, kind="ExternalInput").ap()
    idn = nc.dram_tensor("idn", [128, 128], F32, kind="ExternalInput").ap()
    y = nc.dram_tensor("y", [ROWS_PER_CORE, SIZE], F32, kind="ExternalOutput").ap()

    with tile.TileContext(nc) as tc:
        with (
            tc.tile_pool(name="const", bufs=1) as cpool,
            tc.tile_pool(name="xin", bufs=3) as xpool,
            tc.tile_pool(name="t1", bufs=2) as tpool,
            tc.tile_pool(name="u1", bufs=2) as upool,
            tc.tile_pool(name="v1", bufs=2) as vpool,
            tc.tile_pool(name="w_1", bufs=2) as wpool,
            tc.tile_pool(name="z1", bufs=2) as zpool,
            tc.tile_pool(name="yout", bufs=3) as ypool,
            tc.tile_pool(name="ps", bufs=4, space="PSUM") as pspool,
        ):
            w1_sb = cpool.tile([128, 128], F32, tag="w1")
            w2_sb = cpool.tile([128, 128], F32, tag="w2")
            id_sb = cpool.tile([128, 128], F32, tag="idn")
            nc.sync.dma_start(out=w1_sb[:, :], in_=w1)
            nc.sync.dma_start(out=w2_sb[:, :], in_=w2)
            nc.sync.dma_start(out=id_sb[:, :], in_=idn)

            for k in range(N_TILES):
                X = xpool.tile([128, 2048], F32, tag="X")
                for s in range(2):
                    r0 = k * TILE_ROWS + s * 128
                    nc.sync.dma_start(out=X[:, s * 1024:(s + 1) * 1024],
                                      in_=x[r0:r0 + 128, :])

                T = tpool.tile([128, 2048], F32, tag="T")
                U = upool.tile([128, 2048], F32, tag="U")
                V = vpool.tile([128, 2048], F32, tag="V")
                W = wpool.tile([128, 2048], F32, tag="W")
                Z = zpool.tile([128, 2048], F32, tag="Z")
                Y = ypool.tile([128, 2048], F32, tag="Y")

                for s in range(2):
                    so = s * 1024
                    # A: 8 contiguous 128x128 transposes -> [e7, (g, r)]
                    psA = pspool.tile([128, 1024], F32, tag="ps")
                    for g in range(8):
                        nc.tensor.transpose(
                            psA[:, g * 128:(g + 1) * 128],
                            X[:, so + g * 128: so + (g + 1) * 128],
                            id_sb[:, :])
                    nc.scalar.copy(out=T[:, so:so + 1024], in_=psA[:, :])

                    # B: kron(H8,I16) on elem bits 6..4 (layout preserved)
                    psB = pspool.tile([128, 1024], F32, tag="ps")
                    for j in range(2):
                        nc.tensor.matmul(
                            psB[:, j * 512:(j + 1) * 512],
                            w1_sb[:, :],
                            T[:, so + j * 512: so + (j + 1) * 512])
                    nc.vector.tensor_copy(out=U[:, so:so + 1024], in_=psB[:, :])

                    # C: transpose back -> row-major intermediate V
                    psC = pspool.tile([128, 1024], F32, tag="ps")
                    for g in range(8):
                        nc.tensor.transpose(
                            psC[:, g * 128:(g + 1) * 128],
                            U[:, so + g * 128: so + (g + 1) * 128],
                            id_sb[:, :])
                    csrc = psC[:, :].rearrange(
                        "p (g m l) -> p m g l", g=8, m=8, l=16)
                    cdst = V[:, so:so + 1024].rearrange(
                        "p (m g l) -> p m g l", m=8, g=8, l=16)
                    nc.scalar.copy(out=cdst, in_=csrc)

                    # D: contiguous transposes (gather was folded into copy C)
                    psD = pspool.tile([128, 1024], F32, tag="ps")
                    for m in range(8):
                        nc.tensor.transpose(
                            psD[:, m * 128:(m + 1) * 128],
                            V[:, so + m * 128: so + (m + 1) * 128],
                            id_sb[:, :])
                    nc.vector.tensor_copy(out=W[:, so:so + 1024], in_=psD[:, :])

                    # E: H128 on (g, l4) = elem bits 9..7 and 3..0
                    psE = pspool.tile([128, 1024], F32, tag="ps")
                    for j in range(2):
                        nc.tensor.matmul(
                            psE[:, j * 512:(j + 1) * 512],
                            w2_sb[:, :],
                            W[:, so + j * 512: so + (j + 1) * 512])
                    nc.scalar.copy(out=Z[:, so:so + 1024], in_=psE[:, :])

                    # F: transpose back; scatter (m,g,l)->(g,m,l) in the copy
                    psF = pspool.tile([128, 1024], F32, tag="ps")
                    for m in range(8):
                        nc.tensor.transpose(
                            psF[:, m * 128:(m + 1) * 128],
                            Z[:, so + m * 128: so + (m + 1) * 128],
                            id_sb[:, :])
                    src = psF[:, :].rearrange("p (m g l) -> p m g l", m=8, g=8, l=16)
                    dst = Y[:, so:so + 1024].rearrange(
                        "p (g m l) -> p m g l", g=8, m=8, l=16)
                    nc.vector.tensor_copy(out=dst, in_=src)

                for s in range(2):
                    r0 = k * TILE_ROWS + s * 128
                    nc.sync.dma_start(out=y[r0:r0 + 128, :],
                                      in_=Y[:, s * 1024:(s + 1) * 1024])

    nc.compile()
    return nc


def _get_program():
    if "nc" not in _CACHE:
        _CACHE["nc"] = _build()
    return _CACHE["nc"]


def kernel(x, scale, shift):
    from concourse.bass_utils import run_bass_kernel_spmd

    x = np.asarray(x)
    scale = np.asarray(scale, dtype=np.float32)
    shift = np.asarray(shift, dtype=np.float32)
    orig_shape = x.shape
    xf = np.ascontiguousarray(x.reshape(-1, SIZE).astype(np.float32))

    uniform = np.all(scale == scale[0])
    g = float(scale[0]) / 32.0 if uniform else 1.0 / 32.0

    h8 = _hadamard(8)
    w1 = (np.kron(h8, np.eye(16)) * g).astype(np.float32)
    j = np.arange(128)
    w2 = ((-1.0) ** np.array(
        [[bin(a & b).count("1") for b in j] for a in j])).astype(np.float32)
    idn = np.eye(128, dtype=np.float32)

    nc = _get_program()
    in_maps = []
    for c in range(N_CORES):
        in_maps.append({
            "x": xf[c * ROWS_PER_CORE:(c + 1) * ROWS_PER_CORE],
            "w1": w1, "w2": w2, "idn": idn,
        })
    res = run_bass_kernel_spmd(nc, in_maps, core_ids=list(range(N_CORES)))
    out = np.concatenate([res.results[c]["y"] for c in range(N_CORES)], axis=0)

    if not uniform:
        out = out * scale[None, :]
    if np.any(shift != 0):
        out = out + shift[None, :]
    return out.reshape(orig_shape).astype(x.dtype)
